# revision 5
# baseline (speedup 1.0000x reference)
"""Trainium2 Bass kernel for BaseBidirectionalAttention — round 2.

Shapes: B=32, C=1024, Q=128, D=256, F=4D=1024.  Data-parallel over batch
across 8 cores (4 elems/core); weights replicated.

Structural optimizations over the two-matmul reference:
 1. No nonlinearity between the two linears -> collapse:
        out = relu(mask * (att @ Wc.T + bc)),  Wc = W2@W1, bc = W2@b1+b2
    (host-precomputed once).
 2. att = [ctx, c2q, ctx*c2q, ctx*q2c] block-splits Wc: out =
        ctx @ Wc1.T + c2q @ Wc2.T + (ctx*c2q) @ Wc3.T + (ctx*q2c) @ Wc4.T
    a. q2c is rank-1 over c  ->  (ctx*q2c)@Wc4.T = ctx @ (Wc4 diag(q2c)).T,
       merged into the ctx term with a per-elem fused DVE weight fix-up:
       wmod = Wc1.T + q2c * Wc4.T              (2 scalar_tensor_tensor ops)
    b. c2q = P @ question is rank-Q  ->  c2q@Wc2.T = P @ (question@Wc2.T);
       question@Wc2.T is QxF (cheap), P.T is already materialized for c2q.
    Contraction per output chunk: 2 k-tiles (ctx&q2c merged) + 2 (ctx*c2q)
    + 1 (P, q-dim) = 5 of the naive 8.
 3. Context arrives host-transposed (B,D,C): no on-device ctx transposes,
    and q2c's weighted sum over c runs as a fused DVE multiply-reduce
    against a PE-broadcast of the q2c softmax weights.
 4. Elem-level software pipeline: stage A (sim/softmax/pieces) of elem b+1
    is emitted before stage B (big linear) of elem b, so A's DVE/ACT
    chains hide under B's PE stream.

Heavy matmuls run in float32r (full-rate PE streaming); softmax statistics,
exp, and reductions stay fp32.
"""

import sys

if "/opt/trn_rl_repo" not in sys.path:
    sys.path.insert(0, "/opt/trn_rl_repo")

import numpy as np

import concourse.bass as bass
import concourse.mybir as mybir
import concourse.tile as tile
from concourse import bacc
from concourse.bass_utils import run_bass_kernel_spmd
from concourse.masks import make_identity

B, C, Q, D = 32, 1024, 128, 256
F = 4 * D
NCORES = 8
BPC = B // NCORES  # batch elems per core
P = 128
CT = C // P   # 8 c-tiles
FT = F // P   # 8 f-tiles
DH = D // P   # 2 halves of D
FH = F // 512  # 2 f'-chunks of 512

FP32 = mybir.dt.float32
FP32R = mybir.dt.float32r
AX = mybir.AxisListType.X
AF = mybir.ActivationFunctionType
MUL = mybir.AluOpType.mult
ADD = mybir.AluOpType.add


def _f(ap):
    """fp32 view of a float32r AP (same bits) for DVE/fp32-matmul reads."""
    return ap.bitcast(FP32)


def _build_body(es, tc, outs, ins, n_elems=BPC, reps=1):
    nc = tc.nc
    ctxt_d, qstn_d, qstt_d, vecsT_d, wct_d, bcr_d, mT_d, sel_d = ins
    out_d = outs[0]

    const = es.enter_context(tc.tile_pool(name="const", bufs=1))
    weights = es.enter_context(tc.tile_pool(name="weights", bufs=1))
    loads = es.enter_context(tc.tile_pool(name="loads", bufs=3))
    work = es.enter_context(tc.tile_pool(name="work", bufs=1))
    outp = es.enter_context(tc.tile_pool(name="outp", bufs=4))
    psA = es.enter_context(tc.tile_pool(name="psA", bufs=6, space="PSUM"))
    psB = es.enter_context(tc.tile_pool(name="psB", bufs=2, space="PSUM"))

    # ---- constants / replicated weights ----
    ident = const.tile([P, P], FP32)
    make_identity(nc, ident)
    ones_row = const.tile([1, P], FP32)
    nc.vector.memset(ones_row, 1.0)
    ones_col = const.tile([P, 1], FP32)
    nc.vector.memset(ones_col, 1.0)
    ones512 = const.tile([P, 512], FP32)
    nc.vector.memset(ones512, 1.0)
    selbc = const.tile([CT, CT * P], FP32R)  # sel[t, t*128+i]=1 broadcast lhsT
    vecsr = const.tile([P, DH, 1], FP32R)  # wq column in fp32r (full-rate qwq matmul)
    ones_row_r = const.tile([1, P], FP32R)
    zeros512 = const.tile([P, 512], FP32)
    nc.vector.memset(zeros512, 0.0)

    def load_elem(b, idx):
        ctxT = loads.tile([P, DH, C], FP32R, tag="ctxT", name=f"ctxT{idx}")
        src = ctxt_d[b].rearrange("(h p) c -> p h c", p=P)
        nc.sync.dma_start(ctxT[:, 0], src[:, 0])
        nc.sync.dma_start(ctxT[:, 1], src[:, 1])
        qn = loads.tile([P, D], FP32R, tag="qst_nat", name=f"qst_nat{idx}")
        nc.sync.dma_start(qn[:], qstn_d[b])
        qt = loads.tile([P, DH, P], FP32R, tag="qstT", name=f"qstT{idx}")
        nc.sync.dma_start(qt[:], qstt_d[b].rearrange("(h p) q -> p h q", p=P))
        return ctxT, qn, qt

    # elem-0 loads go before the big weight DMAs (single-shot only: with a
    # For_i timing loop the hoisted tile's slot would be recycled in-loop)
    pend = load_elem(0, 0) if reps == 1 else None

    vecsT = const.tile([P, DH, 3], FP32)  # [p, h, v]: wq/wc/wm at e=h*128+p
    nc.sync.dma_start(vecsT[:], vecsT_d.rearrange("(h p) v -> p h v", p=P))
    nc.sync.dma_start(selbc[:], sel_d)
    nc.vector.tensor_copy(vecsr[:], vecsT[:, :, 0:1])
    nc.vector.tensor_copy(ones_row_r[:], ones_row[:])

    wct = weights.tile([P, FT, F], FP32R)  # [kl, k, f'] = Wc[f', k*128+kl]
    nc.sync.dma_start(wct[:], wct_d.rearrange("(k p) f -> p k f", p=P))
    bcbc = const.tile([P, F], FP32)  # bc broadcast to all partitions
    nc.gpsimd.dma_start(
        out=bcbc[:],
        in_=bass.AP(tensor=bcr_d.tensor, offset=bcr_d.offset, ap=[[0, P]] + bcr_d.ap[1:]),
    )
    mT = const.tile([P, n_elems * CT], FP32)  # [p, b*8+t] = mask[b, t*128+p]
    nc.sync.dma_start(mT[:], mT_d)

    if reps > 1:
        es.enter_context(tc.For_i(0, reps, 1))

    def stage_a1(elem, idx):
        """sim + softmax (and qW2, which has no in-elem deps)."""
        ctxT, qst_nat, qstT = elem
        st = {"elem": elem, "idx": idx}

        # -- tiny preamble: (q*wm)^T (col 128 = w_context, so the sim matmul
        # also yields the cwc column for free) and the qwq broadcast row --
        qmT = work.tile([P, DH, P + 2], FP32R, tag="qmT", bufs=2, name=f"qmT{idx}")
        for dh in range(DH):
            nc.vector.tensor_scalar_mul(qmT[:, dh, :P], _f(qstT[:, dh, :]),
                                        vecsT[:, dh, 2:3])
            nc.vector.tensor_copy(qmT[:, dh, P:P + 2], vecsT[:, dh, 1:3])
        qwq = work.tile([1, P], FP32R, tag="qwq", bufs=2, name=f"qwq{idx}")
        pw = psB.tile([1, P], FP32, tag="ps_small", name=f"pw{idx}")
        for dh in range(DH):
            nc.tensor.matmul(
                pw[:], vecsr[:, dh, :], qstT[:, dh, :],
                start=(dh == 0), stop=(dh == DH - 1),
            )
        nc.vector.tensor_copy(qwq[:], pw[:])
        pqb = psA.tile([P, P], FP32, tag="ps_mm", name=f"pqb{idx}")
        nc.tensor.matmul(pqb[:], ones_row_r[:], qwq[:], start=True, stop=True)
        qwqbc = work.tile([P, P], FP32, tag="qwqbc", bufs=2, name=f"qwqbc{idx}")
        nc.vector.tensor_copy(qwqbc[:], pqb[:])

        # -- sim tiles + softmax over q (free dim) --
        nmx = work.tile([P, CT], FP32, tag="nmx", bufs=2)
        Pm = work.tile([P, CT, P], FP32, tag="Pm", bufs=2)
        sume = work.tile([P, CT], FP32, tag="sume", bufs=2)
        rs = work.tile([P, CT], FP32, tag="rs", bufs=2)
        pcw = work.tile([P, CT], FP32, tag="pcw", bufs=2)  # cwc cols [c_l, t]
        for t in range(CT):
            ps = psB.tile([P, P + 2], FP32, tag="ps_small")
            for dh in range(DH):
                nc.tensor.matmul(
                    ps[:], ctxT[:, dh, t * P:(t + 1) * P], qmT[:, dh, :],
                    start=(dh == 0), stop=(dh == DH - 1),
                )
            nc.vector.tensor_copy(pcw[:, t:t + 1], ps[:, P:P + 1])
            scr = work.tile([P, P], FP32, tag="scr", bufs=2)
            nc.vector.tensor_add(scr[:], ps[:, :P], qwqbc[:])
            nc.vector.reduce_max(nmx[:, t:t + 1], scr[:], axis=AX, negate=True)
            nc.scalar.activation(
                Pm[:, t, :], scr[:], AF.Exp, bias=nmx[:, t:t + 1],
                accum_out=sume[:, t:t + 1],
            )
            nc.vector.reciprocal(rs[:, t:t + 1], sume[:, t:t + 1])
            nc.vector.tensor_scalar_mul(Pm[:, t, :], _f(Pm[:, t, :]),
                                        rs[:, t:t + 1])

        # -- qW2 = question @ Wc2.T + bc  (Q x F).  Rows of P sum to 1, so
        # adding bc here injects the output bias through the P @ qW2 matmul
        # for free -- no per-chunk bias add in stage B.
        qW2 = work.tile([P, F], FP32R, tag="qW2", bufs=2)
        for fh in range(FH):
            pq2 = psA.tile([P, 512], FP32, tag="ps_mm", name=f"pq2{idx}{fh}")
            for dh in range(DH):
                nc.tensor.matmul(
                    pq2[:], qstT[:, dh, :P], wct[:, 2 + dh, fh * 512:(fh + 1) * 512],
                    start=(dh == 0), stop=(dh == DH - 1),
                )
            nc.vector.tensor_add(qW2[:, fh * 512:(fh + 1) * 512], pq2[:],
                                 bcbc[:, fh * 512:(fh + 1) * 512])

        st.update(Pm=Pm, nmx=nmx, pcw=pcw, qW2=qW2)
        return st

    def _nofill(n=1):
        pass

    def stage_a2(st, fill=_nofill):
        """P^T, c2q^T, cxcT = ctx^T * c2q^T."""
        ctxT, qst_nat, qstT = st["elem"]
        Pm = st["Pm"]
        PT = work.tile([P, C], FP32R, tag="PT", bufs=2)
        c2qT = work.tile([P, DH, C], FP32R, tag="c2qT", bufs=2)
        for g in range(2):
            pt = psA.tile([P, 512], FP32, tag="ps_mm")
            for j in range(4):
                t = g * 4 + j
                nc.tensor.transpose(pt[:, j * P:(j + 1) * P],
                                    Pm[:, t, :], ident[:])
            nc.scalar.copy(PT[:, g * 512:(g + 1) * 512], pt[:])
            fill(1)
            for dh in range(DH):
                pc2 = psA.tile([P, 512], FP32, tag="ps_mm", name=f"pc2{g}{dh}")
                nc.tensor.matmul(
                    pc2[:], qst_nat[:, dh * P:(dh + 1) * P],
                    PT[:, g * 512:(g + 1) * 512],
                    start=True, stop=True,
                )
                nc.vector.tensor_copy(c2qT[:, dh, g * 512:(g + 1) * 512], pc2[:])
            fill(1)

        # -- cxcT = ctxT * c2qT (Pool: SBUF-only, keeps DVE free) --
        cxcT = work.tile([P, DH, C], FP32R, tag="cxcT", bufs=2)
        for dh in range(DH):
            nc.gpsimd.tensor_mul(cxcT[:, dh, :], _f(ctxT[:, dh, :]),
                                 _f(c2qT[:, dh, :]))
        st.update(PT=PT, cxcT=cxcT)
        return st

    def stage_a3(st, fill=_nofill):
        """q2c (wall softmax over c) and the merged ctx weights wmod."""
        ctxT, qst_nat, qstT = st["elem"]
        idx, pcw, nmx = st["idx"], st["pcw"], st["nmx"]
        # wall[c_l, t] = exp(madj - Mglob)
        madj = work.tile([P, CT], FP32, tag="madj", bufs=2)
        nc.vector.tensor_sub(madj[:], pcw[:], nmx[:])
        colmin = work.tile([P, 1], FP32, tag="colmin", bufs=2)
        nc.vector.reduce_max(colmin[:], madj[:], axis=AX, negate=True)
        pcm = psB.tile([1, P], FP32, tag="ps_small")
        nc.tensor.transpose(pcm[:], colmin[:], ident[:])
        minall = work.tile([1, 2], FP32, tag="minall", bufs=2)
        nc.vector.tensor_reduce(minall[:, 0:1], pcm[:], axis=AX, op=mybir.AluOpType.min)
        pmb = psB.tile([P, 1], FP32, tag="ps_small")
        nc.tensor.matmul(pmb[:], ones_row[:], minall[:, 0:1], start=True, stop=True)
        minb = work.tile([P, 1], FP32, tag="minb", bufs=2)
        nc.vector.tensor_copy(minb[:], pmb[:])
        wall = work.tile([P, CT], FP32, tag="wall", bufs=2)
        nc.scalar.activation(wall[:], madj[:], AF.Exp, bias=minb[:])
        fill(1)

        # numerator + denominator: wall broadcast to all partitions via a
        # block-diagonal [t, t*128+c_l] tile (off-diag zeros) contracted
        # against all-ones -- a full-rate 512-wide PE matmul per chunk.
        # Fused DVE multiply+reduce passes then accumulate num[d_l, dh] =
        # sum_c ctxT*wallbc and den = sum_c wallbc (ones input).
        pwT = psB.tile([CT, P], FP32, tag="ps_small")
        nc.tensor.transpose(pwT[:], wall[:], ident[:])  # [t, c_l]
        wallT = work.tile([CT, P], FP32R, tag="wallT", bufs=2)
        nc.vector.tensor_copy(wallT[:], pwT[:])
        fill(1)
        q2cp = work.tile([P, 2 * DH + 2], FP32, tag="q2cp", bufs=2)
        junk = work.tile([P, 512], FP32, tag="junk")
        for g in range(2):
            ws = slice(g * 512, (g + 1) * 512)
            pbc = psA.tile([P, 512], FP32, tag="ps_mm", name=f"pbc{idx}{g}")
            for j in range(4):
                t = g * 4 + j
                nc.tensor.matmul(pbc[:, j * P:(j + 1) * P],
                                 selbc[:, t * P:(t + 1) * P], wallT[:],
                                 start=True, stop=True)
            for dh in range(DH):
                nc.vector.scalar_tensor_tensor(
                    junk[:], _f(ctxT[:, dh, ws]), 1.0, pbc[:],
                    op0=MUL, op1=MUL,
                    accum_out=q2cp[:, g * DH + dh:g * DH + dh + 1],
                )
            nc.vector.scalar_tensor_tensor(
                junk[:], ones512[:], 1.0, pbc[:],
                op0=MUL, op1=MUL,
                accum_out=q2cp[:, 2 * DH + g:2 * DH + g + 1],
            )
            fill(1)
        q2cn = work.tile([P, DH], FP32, tag="q2cn", bufs=2)
        for dh in range(DH):
            nc.vector.tensor_add(q2cn[:, dh:dh + 1], q2cp[:, dh:dh + 1],
                                 q2cp[:, DH + dh:DH + dh + 1])
        den = work.tile([P, 1], FP32, tag="den", bufs=2)
        nc.vector.tensor_add(den[:], q2cp[:, 2 * DH:2 * DH + 1],
                             q2cp[:, 2 * DH + 1:2 * DH + 2])
        rdenb = work.tile([P, 1], FP32, tag="rdenb", bufs=2)
        nc.vector.reciprocal(rdenb[:], den[:])
        q2cf = work.tile([P, DH], FP32, tag="q2cf", bufs=2)
        nc.vector.tensor_scalar_mul(q2cf[:], q2cn[:], rdenb[:])

        # -- merged ctx weights: wmod = Wc1.T + q2c * Wc4.T (per elem) --
        wmod = work.tile([P, DH, F], FP32R, tag="wmod", bufs=2)
        for dh in range(DH):
            nc.vector.scalar_tensor_tensor(
                wmod[:, dh, :], _f(wct[:, 6 + dh, :]), q2cf[:, dh:dh + 1],
                _f(wct[:, dh, :]), op0=MUL, op1=ADD,
            )
        st.update(wmod=wmod)
        return st

    def emit_chunk(b, st, m, osbs):
        """Chunk m = (ct, fh) of the collapsed linear + mask + relu + store.
        PSUM eviction alternates ACT (relu w/ mask scale) and Pool (fused
        mask-mul + max-with-zero) so neither engine's burst blocks PE."""
        ctxT, qst_nat, qstT = st["elem"]
        cxcT, PT, qW2, wmod = st["cxcT"], st["PT"], st["qW2"], st["wmod"]
        ct, fh = m // FH, m % FH
        if fh == 0:
            osbs[ct] = outp.tile([P, F], FP32, tag="osb", name=f"osb{b}_{ct}")
        osb = osbs[ct]
        cs = slice(ct * P, (ct + 1) * P)
        fs = slice(fh * 512, (fh + 1) * 512)
        p2 = psA.tile([P, 512], FP32, tag="ps_mm")
        nc.tensor.matmul(p2[:], PT[:, cs], qW2[:, fs], start=True, stop=False)
        for dh in range(DH):
            nc.tensor.matmul(p2[:], cxcT[:, dh, cs], wct[:, 4 + dh, fs],
                             start=False, stop=False)
        for dh in range(DH):
            nc.tensor.matmul(p2[:], ctxT[:, dh, cs], wmod[:, dh, fs],
                             start=False, stop=(dh == DH - 1))
        msc = mT[:, b * CT + ct:b * CT + ct + 1]
        nc.scalar.activation(osb[:, fs], p2[:], AF.Relu, scale=msc)
        nc.sync.dma_start(out_d[b, cs, fs], osb[:, fs])

    # ---- elem-level software pipeline.  A(b+1)'s three phases are emitted
    # between fills of B(b)'s 16 linear chunks, so the in-order PE queue
    # always has B matmuls to run while A's cross-engine softmax / q2c
    # chains drain on DVE/ACT. ----
    NCH = CT * FH
    if pend is None:
        pend = load_elem(0, 0)
    nxt = load_elem(1, 1) if n_elems > 1 else None
    st = stage_a3(stage_a2(stage_a1(pend, 0)))
    for b in range(n_elems):
        load_nxt = load_elem(b + 2, b + 2) if b + 2 < n_elems else None
        osbs = {}
        mstate = [0]

        def fill(n, b=b, st=st, osbs=osbs, mstate=mstate):
            hi = min(mstate[0] + n, NCH)
            for m in range(mstate[0], hi):
                emit_chunk(b, st, m, osbs)
            mstate[0] = hi

        stn = stage_a1(nxt, b + 1) if nxt is not None else None
        fill(2)
        if stn is not None:
            stn = stage_a2(stn, fill)
        fill(2)
        if stn is not None:
            stn = stage_a3(stn, fill)
        fill(NCH)
        st, nxt = stn, load_nxt


_NC_CACHE = {}


def _build_nc(n_elems=BPC, reps=1):
    key = (n_elems, reps)
    if key in _NC_CACHE:
        return _NC_CACHE[key]
    nc = bacc.Bacc("TRN2", target_bir_lowering=False, debug=False, num_devices=NCORES)
    ins = [
        nc.dram_tensor("ctxt", (n_elems, D, C), FP32R, kind="ExternalInput").ap(),
        nc.dram_tensor("qstn", (n_elems, Q, D), FP32R, kind="ExternalInput").ap(),
        nc.dram_tensor("qstt", (n_elems, D, Q), FP32R, kind="ExternalInput").ap(),
        nc.dram_tensor("vecsT", (D, 3), FP32, kind="ExternalInput").ap(),
        nc.dram_tensor("wct", (F, F), FP32R, kind="ExternalInput").ap(),
        nc.dram_tensor("bcr", (1, F), FP32, kind="ExternalInput").ap(),
        nc.dram_tensor("mT", (P, n_elems * CT), FP32, kind="ExternalInput").ap(),
        nc.dram_tensor("sel", (CT, CT * P), FP32R, kind="ExternalInput").ap(),
    ]
    outs = [nc.dram_tensor("out", (n_elems, C, F), FP32, kind="ExternalOutput").ap()]
    from contextlib import ExitStack
    with tile.TileContext(nc) as tc, ExitStack() as es:
        _build_body(es, tc, outs, ins, n_elems=n_elems, reps=reps)
    nc.compile()
    _NC_CACHE[key] = (nc, ins, outs)
    return _NC_CACHE[key]


def _host_prep(context, question, context_mask, w_question, w_context, w_multiple,
               W1, b1, W2, b2):
    """Build the 8 per-core input maps from full inputs."""
    context = np.asarray(context, np.float32)
    question = np.asarray(question, np.float32)
    maskf = np.asarray(context_mask).astype(np.float32)
    vecsT = np.ascontiguousarray(
        np.stack([w_question, w_context, w_multiple]).T.astype(np.float32))  # (D,3)
    W1 = np.asarray(W1, np.float32)
    W2 = np.asarray(W2, np.float32)
    Wc = W2 @ W1                      # collapsed linear (no nonlinearity between)
    bc = W2 @ np.asarray(b1, np.float32) + np.asarray(b2, np.float32)
    wct = np.ascontiguousarray(Wc.T)  # [k, f'] = Wc[f', k]
    bcr = bc.reshape(1, F)
    sel = np.zeros((CT, CT * P), np.float32)
    for t in range(CT):
        sel[t, t * P:(t + 1) * P] = 1.0
    in_maps = []
    for i in range(NCORES):
        sl = slice(BPC * i, BPC * (i + 1))
        mTc = np.ascontiguousarray(
            maskf[sl].reshape(BPC, CT, P).transpose(2, 0, 1).reshape(P, BPC * CT))
        in_maps.append({
            "ctxt": np.ascontiguousarray(context[sl].transpose(0, 2, 1)),
            "qstn": np.ascontiguousarray(question[sl]),
            "qstt": np.ascontiguousarray(question[sl].transpose(0, 2, 1)),
            "vecsT": vecsT,
            "wct": wct,
            "bcr": bcr,
            "mT": mTc,
            "sel": sel,
        })
    return in_maps


def kernel(context, question, context_mask, w_question, w_context, w_multiple,
           W1, b1, W2, b2):
    nc, _, _ = _build_nc()
    in_maps = _host_prep(context, question, context_mask, w_question, w_context,
                         w_multiple, W1, b1, W2, b2)
    res = run_bass_kernel_spmd(nc, in_maps, list(range(NCORES))).results
    out = np.concatenate([res[i]["out"] for i in range(NCORES)], axis=0)
    return out
